# revision 1
# baseline (speedup 1.0000x reference)
# GQA attention block on 8 Trainium2 NeuronCores.
# Sharding: core = (batch b in {0,1}) x (tensor-parallel t in {0..3}).
# Each core: batch row b, 4 query heads {4t..4t+3}, 2 kv heads {2t, 2t+1}.
# W_Q/W_K/W_V split column-wise (per-head), W_O row-wise; the 4 TP partial
# outputs per batch are summed on the host (the "all-reduce").
import math
import sys

sys.path.insert(0, "/opt/trn_rl_repo")

import ml_dtypes
import numpy as np

import concourse.bacc as bacc
import concourse.bass as bass
import concourse.mybir as mybir
import concourse.tile as tile
from contextlib import ExitStack

BF = mybir.dt.bfloat16
F32 = mybir.dt.float32
bfnp = ml_dtypes.bfloat16

EMB = 2048
HEADS = 16
G = 2
HD = 128          # head dim
KV = HEADS // G   # 8 kv heads
B = 2
S = 2048
NCORES = 8
TP = 4
HQ = HEADS // TP       # 4 q heads per core
HKV = KV // TP         # 2 kv heads per core
NE = EMB // 128        # 16 contraction chunks
SC4 = S // 512         # 4 s-chunks of 512
SC16 = S // 128        # 16 s-chunks of 128
SCALE = 1.0 / math.sqrt(float(EMB))

_NC = None


def _build_program(loop_n=None):
    nc = bacc.Bacc("TRN2", target_bir_lowering=False, debug=False)

    xT = nc.dram_tensor("xT", (EMB, S), BF, kind="ExternalInput")
    wq = nc.dram_tensor("wq", (EMB, HQ * HD), BF, kind="ExternalInput")
    wk = nc.dram_tensor("wk", (EMB, HKV * HD), BF, kind="ExternalInput")
    wv = nc.dram_tensor("wv", (EMB, HKV * HD), BF, kind="ExternalInput")
    wo = nc.dram_tensor("wo", (HQ * HD, EMB), BF, kind="ExternalInput")
    cosT = nc.dram_tensor("cosT", (HD, S), F32, kind="ExternalInput")
    sinT = nc.dram_tensor("sinT", (HD, S), F32, kind="ExternalInput")
    out = nc.dram_tensor("out", (S, EMB), F32, kind="ExternalOutput")

    with tile.TileContext(nc) as tc, ExitStack() as ctx:
        persist = ctx.enter_context(tc.tile_pool(name="persist", bufs=1))
        # qk_sb j-blocks: 0..3 = roped Q heads, 4..5 = roped K kv-heads; [d, s]
        qk_sb = persist.tile([128, HQ + HKV, S], BF)
        # V in [t, d] layout: [t_part, t_chunk, kvl*128+d]
        v_sb = persist.tile([128, SC16, HKV * HD], BF)
        ctx_sb = persist.tile([128, HQ, S], BF)      # [d, head, s]
        wo_sb = persist.tile([128, HQ, EMB], BF)     # [d, head, e_out]
        ones_sb = persist.tile([128, 1], BF)
        nc.vector.memset(ones_sb, 1.0)
        for jb in range(HQ):
            nc.sync.dma_start(out=wo_sb[:, jb, :], in_=wo[jb * 128:(jb + 1) * 128, :])

        def _phases():
            # ---------------- Phase 1: projections + RoPE ----------------
            with tc.tile_pool(name="xt", bufs=1) as xt_pool, \
                 tc.tile_pool(name="wts", bufs=1) as w_pool, \
                 tc.tile_pool(name="ropet", bufs=4) as rope_t, \
                 tc.tile_pool(name="pproj", bufs=8, space=bass.MemorySpace.PSUM) as pp:
                xt_sb = xt_pool.tile([128, NE, S], BF)
                for c in range(NE):
                    nc.sync.dma_start(out=xt_sb[:, c, :], in_=xT[c * 128:(c + 1) * 128, :])
                wq_sb = w_pool.tile([128, NE, HQ * HD], BF)
                wk_sb = w_pool.tile([128, NE, HKV * HD], BF)
                wv_sb = w_pool.tile([128, NE, HKV * HD], BF)
                cos_sb = w_pool.tile([128, S], F32)
                sin_sb = w_pool.tile([128, S], F32)
                for c in range(NE):
                    nc.sync.dma_start(out=wq_sb[:, c, :], in_=wq[c * 128:(c + 1) * 128, :])
                    nc.sync.dma_start(out=wk_sb[:, c, :], in_=wk[c * 128:(c + 1) * 128, :])
                    nc.sync.dma_start(out=wv_sb[:, c, :], in_=wv[c * 128:(c + 1) * 128, :])
                nc.sync.dma_start(out=cos_sb, in_=cosT[:, :])
                nc.sync.dma_start(out=sin_sb, in_=sinT[:, :])

                # Q/K projection in transposed [d, s] layout + RoPE
                def do_qk(jb):
                    pts = []
                    for sc in range(SC4):
                        pts.append(pp.tile([128, 512], F32, tag="pts", name=f"pts_{jb}_{sc}"))
                    for c in range(NE):
                        if jb < HQ:
                            lhsT = wq_sb[:, c, jb * 128:(jb + 1) * 128]
                        else:
                            kvl = jb - HQ
                            lhsT = wk_sb[:, c, kvl * 128:(kvl + 1) * 128]
                        for sc in range(SC4):
                            nc.tensor.matmul(
                                pts[sc], lhsT, xt_sb[:, c, sc * 512:(sc + 1) * 512],
                                start=(c == 0), stop=(c == NE - 1),
                            )
                    for sc in range(SC4):
                        sl = slice(sc * 512, (sc + 1) * 512)
                        xs = rope_t.tile([128, 512], F32, tag="xs")
                        nc.scalar.copy(xs, pts[sc])
                        xw = rope_t.tile([128, 512], F32, tag="xw")
                        nc.sync.dma_start(out=xw[0:64, :], in_=xs[64:128, :])
                        nc.sync.dma_start(out=xw[64:128, :], in_=xs[0:64, :])
                        t1 = rope_t.tile([128, 512], F32, tag="t1")
                        nc.vector.tensor_mul(t1, xs, cos_sb[:, sl])
                        nc.vector.tensor_mul(xw, xw, sin_sb[:, sl])
                        nc.vector.tensor_add(qk_sb[:, jb, sl], t1, xw)

                # V in [t, d] layout (no rope): out[t=128, kvl*128+d]
                def do_v():
                    for st in range(SC16):
                        pv = pp.tile([128, 512], F32, tag="pts", name=f"pv_{st}")
                        for c in range(NE):
                            nc.tensor.matmul(
                                pv[:, 0:HKV * HD],
                                xt_sb[:, c, st * 128:(st + 1) * 128],
                                wv_sb[:, c, :],
                                start=(c == 0), stop=(c == NE - 1),
                            )
                        nc.scalar.copy(v_sb[:, st, :], pv[:, 0:HKV * HD])

                # K and V first so attention h=0 can begin while Q1..3 project
                do_qk(HQ)
                do_qk(HQ + 1)
                do_v()
                for jb in range(HQ):
                    do_qk(jb)

            # ---------------- Phase 2: attention ----------------
            with tc.tile_pool(name="pscore", bufs=3, space=bass.MemorySpace.PSUM) as psc, \
                 tc.tile_pool(name="pctx", bufs=2, space=bass.MemorySpace.PSUM) as pcx, \
                 tc.tile_pool(name="pden", bufs=2, space=bass.MemorySpace.PSUM) as pdn, \
                 tc.tile_pool(name="expp", bufs=6) as expp, \
                 tc.tile_pool(name="misc", bufs=2) as misc:
                for h in range(HQ):
                    kvjb = HQ + h // 2     # K block in qk_sb
                    kvl = h // 2           # local kv index into v_sb columns
                    for sc in range(SC4):
                        ssl = slice(sc * 512, (sc + 1) * 512)
                        cps = pcx.tile([128, 512], F32, tag="cps")
                        dps = pdn.tile([1, 512], F32, tag="dps")
                        for tcn in range(SC16):
                            sps = psc.tile([128, 512], F32, tag="sps")
                            nc.tensor.matmul(
                                sps,
                                qk_sb[:, kvjb, tcn * 128:(tcn + 1) * 128],
                                qk_sb[:, h, ssl],
                                start=True, stop=True,
                            )
                            ex = expp.tile([128, 512], BF, tag="ex")
                            nc.scalar.activation(
                                ex, sps, mybir.ActivationFunctionType.Exp, scale=SCALE
                            )
                            nc.tensor.matmul(
                                cps,
                                v_sb[:, tcn, kvl * 128:(kvl + 1) * 128],
                                ex,
                                start=(tcn == 0), stop=(tcn == SC16 - 1),
                            )
                            nc.tensor.matmul(
                                dps, ones_sb, ex,
                                start=(tcn == 0), stop=(tcn == SC16 - 1),
                            )
                        rc = misc.tile([1, 512], F32, tag="rc")
                        nc.vector.reciprocal(rc, dps)
                        rb = misc.tile([128, 512], F32, tag="rb")
                        nc.gpsimd.partition_broadcast(rb, rc)
                        nc.vector.tensor_mul(ctx_sb[:, h, ssl], cps, rb)

            # ---------------- Phase 3: output projection ----------------
            with tc.tile_pool(name="pout", bufs=4, space=bass.MemorySpace.PSUM) as pou, \
                 tc.tile_pool(name="outs", bufs=4) as outp:
                for so in range(SC16):
                    for ec in range(SC4):
                        ops = pou.tile([128, 512], F32, tag="ops")
                        for hl in range(HQ):
                            nc.tensor.matmul(
                                ops,
                                ctx_sb[:, hl, so * 128:(so + 1) * 128],
                                wo_sb[:, hl, ec * 512:(ec + 1) * 512],
                                start=(hl == 0), stop=(hl == HQ - 1),
                            )
                        ot = outp.tile([128, 512], F32, tag="ot")
                        nc.scalar.copy(ot, ops)
                        nc.sync.dma_start(
                            out=out[so * 128:(so + 1) * 128, ec * 512:(ec + 1) * 512],
                            in_=ot,
                        )


        if loop_n is not None:
            with tc.For_i(0, loop_n, 1):
                _phases()
        else:
            _phases()

    nc.compile()
    return nc


def _get_nc():
    global _NC
    if _NC is None:
        _NC = _build_program()
    return _NC


def _rope_tables():
    half = HD // 2
    inv_freq = 1.0 / (10000.0 ** (np.arange(half, dtype=np.float64) * 2.0 / HD))
    ang = np.arange(S, dtype=np.float64)[:, None] * inv_freq[None, :]  # (S, 64)
    cos = np.concatenate([np.cos(ang), np.cos(ang)], axis=1).T  # (128, S)
    sin = np.concatenate([-np.sin(ang), np.sin(ang)], axis=1).T  # pre-signed
    return (np.ascontiguousarray(cos, dtype=np.float32),
            np.ascontiguousarray(sin, dtype=np.float32))


def build_in_maps(x, W_Q, W_K, W_V, W_O):
    x = np.asarray(x, dtype=np.float32)
    W_Q = np.asarray(W_Q, dtype=np.float32)
    W_K = np.asarray(W_K, dtype=np.float32)
    W_V = np.asarray(W_V, dtype=np.float32)
    W_O = np.asarray(W_O, dtype=np.float32)
    cos, sin = _rope_tables()
    in_maps = []
    xTb = [np.ascontiguousarray(x[b].T).astype(bfnp) for b in range(B)]
    for b in range(B):
        for t in range(TP):
            qheads = list(range(HQ * t, HQ * t + HQ))
            kvheads = [HKV * t + i for i in range(HKV)]
            idxq = [d * HEADS + h for h in qheads for d in range(HD)]
            idxkv = [d * KV + kv for kv in kvheads for d in range(HD)]
            rows_o = [h * HD + d for h in qheads for d in range(HD)]
            in_maps.append(dict(
                xT=xTb[b],
                wq=np.ascontiguousarray(W_Q[idxq, :].T).astype(bfnp),
                wk=np.ascontiguousarray(W_K[idxkv, :].T).astype(bfnp),
                wv=np.ascontiguousarray(W_V[idxkv, :].T).astype(bfnp),
                wo=np.ascontiguousarray(W_O[:, rows_o].T).astype(bfnp),
                cosT=cos,
                sinT=sin,
            ))
    return in_maps


def combine_outs(outs):
    out = np.empty((B, S, EMB), dtype=np.float32)
    for b in range(B):
        acc = outs[TP * b].astype(np.float32).copy()
        for t in range(1, TP):
            acc += outs[TP * b + t]
        out[b] = acc
    return out


LAST_RESULTS = None


def kernel(x, W_Q, W_K, W_V, W_O):
    global LAST_RESULTS
    from concourse.bass_utils import run_bass_kernel_spmd

    nc = _get_nc()
    in_maps = build_in_maps(x, W_Q, W_K, W_V, W_O)
    res = run_bass_kernel_spmd(nc, in_maps, list(range(NCORES)))
    LAST_RESULTS = res
    outs = [r["out"] for r in res.results]
    return combine_outs(outs)



# revision 3
# speedup vs baseline: 1.1859x; 1.1859x over previous
# GQA attention block on 8 Trainium2 NeuronCores — restructured bf16 edition.
# Sharding: core = (batch b in {0,1}) x (tensor-parallel t in {0..3}).
# Each core: batch row b, 4 query heads {4t..4t+3}, 2 kv heads {2t, 2t+1}.
# W_Q/W_K/W_V split column-wise (per-head), W_O row-wise; the 4 TP partial
# outputs per batch are summed on the host (the "all-reduce").
#
# vs the naive schedule:
#  - softmax exp batched over PSUM bank-pairs (N=1024 per ACT instruction)
#  - softmax denominator moved off the tensor engine: DVE running adds over
#    the ex tiles + one gpsimd partition_all_reduce (PE saves a full second
#    pass over ex), reciprocal via the fast DVE approximation
#  - output projection interleaved per 512-row block so PE never drains
#  - RoPE in bf16 (2x DVE modes), output written as bf16 (halves out DMA)
import math
import sys

sys.path.insert(0, "/opt/trn_rl_repo")

import ml_dtypes
import numpy as np

import concourse.bacc as bacc
import concourse.bass as bass
import concourse.bass_isa as bass_isa
import concourse.mybir as mybir
import concourse.tile as tile
from contextlib import ExitStack

BF = mybir.dt.bfloat16
F32 = mybir.dt.float32
bfnp = ml_dtypes.bfloat16

EMB = 2048
HEADS = 16
G = 2
HD = 128          # head dim
KV = HEADS // G   # 8 kv heads
B = 2
S = 2048
NCORES = 8
TP = 4
HQ = HEADS // TP       # 4 q heads per core
HKV = KV // TP         # 2 kv heads per core
NE = EMB // 128        # 16 contraction chunks
SC4 = S // 512         # 4 s-chunks of 512
SC16 = S // 128        # 16 s-chunks of 128
SCALE = 1.0 / math.sqrt(float(EMB))

_NC = None


def _build_program(loop_n=None):
    nc = bacc.Bacc("TRN2", target_bir_lowering=False, debug=False)

    xT = nc.dram_tensor("xT", (EMB, S), BF, kind="ExternalInput")
    wq = nc.dram_tensor("wq", (EMB, HQ * HD), BF, kind="ExternalInput")
    wk = nc.dram_tensor("wk", (EMB, HKV * HD), BF, kind="ExternalInput")
    wv = nc.dram_tensor("wv", (EMB, HKV * HD), BF, kind="ExternalInput")
    wo = nc.dram_tensor("wo", (HQ * HD, EMB), BF, kind="ExternalInput")
    cosT = nc.dram_tensor("cosT", (HD, S), BF, kind="ExternalInput")
    sinT = nc.dram_tensor("sinT", (HD, S), BF, kind="ExternalInput")
    out = nc.dram_tensor("out", (S, EMB), BF, kind="ExternalOutput")

    with tile.TileContext(nc) as tc, ExitStack() as ctx:
        persist = ctx.enter_context(tc.tile_pool(name="persist", bufs=1))
        # roped Q (jb 0..3) and K (jb 4..5), bf16: [d, jb, sc, s512]
        qk_sb = persist.tile([128, HQ + HKV, SC4, 512], BF)
        # V in [t, d] layout: [t_part, t_chunk, kvl*128+d]
        v_sb = persist.tile([128, SC16, HKV * HD], BF)
        ctx_sb = persist.tile([128, HQ, SC4, 512], BF)   # [d, head, sc, s]
        wo_sb = persist.tile([128, HQ, SC4, 512], BF)    # [d, head, ec, e]
        xt_sb = persist.tile([128, NE, S], BF)
        wqs = persist.tile([128, NE, HQ * HD], BF)
        wks = persist.tile([128, NE, HKV * HD], BF)
        wvs = persist.tile([128, NE, HKV * HD], BF)
        cos_sb = persist.tile([128, SC4, 512], BF)
        sin_sb = persist.tile([128, SC4, 512], BF)

        # batched input loads: few multi-dim DMAs (the SP sequencer pays
        # ~0.6us dispatch per DMA). xT is split so its completion semaphores
        # fire progressively and the first projection can start early.
        xTr = xT.rearrange("(c p) s -> p c s", p=128)
        nc.sync.dma_start(out=xt_sb[:, 0:2, :], in_=xTr[:, 0:2, :])
        nc.sync.dma_start(out=wks, in_=wk.rearrange("(c p) j -> p c j", p=128))
        nc.sync.dma_start(out=xt_sb[:, 2:4, :], in_=xTr[:, 2:4, :])
        nc.sync.dma_start(out=wvs, in_=wv.rearrange("(c p) j -> p c j", p=128))
        for ci in range(2, 8):
            nc.sync.dma_start(
                out=xt_sb[:, 2 * ci:2 * ci + 2, :], in_=xTr[:, 2 * ci:2 * ci + 2, :]
            )
        nc.sync.dma_start(out=wqs, in_=wq.rearrange("(c p) j -> p c j", p=128))
        nc.sync.dma_start(out=cos_sb, in_=cosT.rearrange("p (sc s) -> p sc s", s=512))
        nc.sync.dma_start(out=sin_sb, in_=sinT.rearrange("p (sc s) -> p sc s", s=512))
        nc.sync.dma_start(
            out=wo_sb, in_=wo.rearrange("(jb p) (ec e) -> p jb ec e", p=128, e=512)
        )

        # PSUM budget (8 banks): pairs 2x2 + accp 2 + oacc 2
        pairs = ctx.enter_context(tc.tile_pool(name="pairs", bufs=2, space="PSUM"))
        accp = ctx.enter_context(tc.tile_pool(name="accp", bufs=2, space="PSUM"))
        oacc = ctx.enter_context(tc.tile_pool(name="oacc", bufs=2, space="PSUM"))
        ropet = ctx.enter_context(tc.tile_pool(name="ropet", bufs=3))
        expool = ctx.enter_context(tc.tile_pool(name="expool", bufs=3))
        dccp = ctx.enter_context(tc.tile_pool(name="dccp", bufs=1))
        darp = ctx.enter_context(tc.tile_pool(name="darp", bufs=1))
        rbp = ctx.enter_context(tc.tile_pool(name="rbp", bufs=1))
        outs = ctx.enter_context(tc.tile_pool(name="outs", bufs=2))

        def _phases():
            # ---------------- Phase 1: projections + RoPE ----------------
            def do_qk(jb):
                if jb < HQ:
                    w_sb, jsl = wqs, slice(jb * 128, (jb + 1) * 128)
                else:
                    kvl = jb - HQ
                    w_sb, jsl = wks, slice(kvl * 128, (kvl + 1) * 128)
                for scp in range(2):      # pairs of 512-wide s-chunks
                    pt = pairs.tile([128, 2, 512], F32, tag="pairs")
                    for c in range(NE):
                        lhsT = w_sb[:, c, jsl]
                        for k in range(2):
                            sck = 2 * scp + k
                            nc.tensor.matmul(
                                pt[:, k, :], lhsT,
                                xt_sb[:, c, sck * 512:(sck + 1) * 512],
                                start=(c == 0), stop=(c == NE - 1),
                            )
                    # RoPE on [128, 2, 512] (both s-chunks at once), bf16
                    xs = ropet.tile([128, 2, 512], BF, tag="xs")
                    nc.scalar.copy(xs, pt)
                    xw = ropet.tile([128, 2, 512], BF, tag="xw")
                    nc.sync.dma_start(out=xw[0:64, :, :], in_=xs[64:128, :, :])
                    nc.sync.dma_start(out=xw[64:128, :, :], in_=xs[0:64, :, :])
                    csl = slice(2 * scp, 2 * scp + 2)
                    nc.vector.tensor_mul(xs, xs, cos_sb[:, csl, :])
                    nc.vector.tensor_mul(xw, xw, sin_sb[:, csl, :])
                    nc.vector.tensor_add(qk_sb[:, jb, csl, :], xs, xw)

            def do_v():
                for st in range(SC16):
                    pv = accp.tile([128, 512], F32, tag="accp")
                    for c in range(NE):
                        nc.tensor.matmul(
                            pv[:, 0:HKV * HD],
                            xt_sb[:, c, st * 128:(st + 1) * 128],
                            wvs[:, c, :],
                            start=(c == 0), stop=(c == NE - 1),
                        )
                    nc.scalar.copy(v_sb[:, st, :], pv[:, 0:HKV * HD])

            # K and V first so attention h=0 can begin while Q1..3 project
            do_qk(HQ)
            do_qk(HQ + 1)
            do_v()
            for jb in range(HQ):
                do_qk(jb)

            # ---------- Phase 2+3: attention + output projection ----------
            # software-pipelined: outproj(sc-1) is EMITTED after
            # attention(sc) so the scheduler prefers the ACT-gated attention
            # stream and uses outproj matmuls/copies as PE/DVE fill work.
            def attention(sc):
                for h in range(HQ):
                    kvjb = HQ + h // 2
                    kvl = h // 2
                    cps = accp.tile([128, 512], F32, tag="accp")
                    dacc = dccp.tile([128, 512], BF, tag="dacc")

                    def scores(g):
                        sp = pairs.tile([128, 2, 512], F32, tag="pairs")
                        for k in range(2):
                            tcn = 2 * g + k
                            nc.tensor.matmul(
                                sp[:, k, :],
                                qk_sb[:, kvjb, tcn // 4, (tcn % 4) * 128:(tcn % 4) * 128 + 128],
                                qk_sb[:, h, sc, :],
                                start=True, stop=True,
                            )
                        return sp

                    # scores run one pair ahead of exp/ctx so the static PE
                    # stream never blocks on the activation latency
                    sp_next = scores(0)
                    for g in range(8):        # pairs of 128-wide t-chunks
                        sp = sp_next
                        if g < 7:
                            sp_next = scores(g + 1)
                        ex = expool.tile([128, 2, 512], BF, tag="ex")
                        nc.scalar.activation(
                            ex, sp, mybir.ActivationFunctionType.Exp, scale=SCALE,
                        )
                        for k in range(2):
                            nc.tensor.matmul(
                                cps,
                                v_sb[:, 2 * g + k, kvl * 128:(kvl + 1) * 128],
                                ex[:, k, :],
                                start=(g == 0 and k == 0), stop=(g == 7 and k == 1),
                            )
                        if g == 0:
                            nc.vector.tensor_add(dacc, ex[:, 0, :], ex[:, 1, :])
                        else:
                            nc.vector.tensor_add(dacc, dacc, ex[:, 0, :])
                            nc.vector.tensor_add(dacc, dacc, ex[:, 1, :])
                    dar = darp.tile([128, 512], F32, tag="dar")
                    nc.gpsimd.partition_all_reduce(
                        dar, dacc, 128, bass_isa.ReduceOp.add
                    )
                    rb = rbp.tile([128, 512], F32, tag="rb")
                    nc.vector.reciprocal_approx_fast(rb, dar)
                    nc.vector.tensor_mul(ctx_sb[:, h, sc, :], cps, rb)
            # output projection for the 4 s-row-chunks of one sc block.
            # On the last block attention is finished: the scores psum
            # pool and the ACT engine are free, so use them for extra
            # pipeline depth there.
            def outproj(sc):
                last = sc == SC4 - 1
                for so4 in range(4):
                    ot4 = outs.tile([128, SC4, 512], BF, tag="ot")
                    for ec in range(SC4):
                        if last and ec % 2 == 0:
                            opsP = pairs.tile([128, 2, 512], F32, tag="pairs")
                        if last:
                            ops = opsP[:, ec % 2, :]
                        else:
                            ops = oacc.tile([128, 512], F32, tag="oacc")
                        for hl in range(HQ):
                            nc.tensor.matmul(
                                ops,
                                ctx_sb[:, hl, sc, so4 * 128:(so4 + 1) * 128],
                                wo_sb[:, hl, ec, :],
                                start=(hl == 0), stop=(hl == HQ - 1),
                            )
                        if last and ec % 2 == 1:
                            nc.scalar.copy(ot4[:, ec, :], ops)
                        else:
                            nc.vector.tensor_copy(ot4[:, ec, :], ops)
                    so = sc * 4 + so4
                    nc.sync.dma_start(
                        out=out[so * 128:(so + 1) * 128, :].rearrange(
                            "p (ec e) -> p ec e", e=512
                        ),
                        in_=ot4,
                    )

            attention(0)
            for sc in range(1, SC4):
                attention(sc)
                outproj(sc - 1)
            outproj(SC4 - 1)

        if loop_n is not None:
            with tc.For_i(0, loop_n, 1):
                _phases()
        else:
            _phases()

    nc.compile()
    return nc


def _get_nc():
    global _NC
    if _NC is None:
        _NC = _build_program()
    return _NC


def _rope_tables():
    half = HD // 2
    inv_freq = 1.0 / (10000.0 ** (np.arange(half, dtype=np.float64) * 2.0 / HD))
    ang = np.arange(S, dtype=np.float64)[:, None] * inv_freq[None, :]  # (S, 64)
    cos = np.concatenate([np.cos(ang), np.cos(ang)], axis=1).T  # (128, S)
    sin = np.concatenate([-np.sin(ang), np.sin(ang)], axis=1).T  # pre-signed
    return (np.ascontiguousarray(cos).astype(bfnp),
            np.ascontiguousarray(sin).astype(bfnp))


def build_in_maps(x, W_Q, W_K, W_V, W_O):
    x = np.asarray(x, dtype=np.float32)
    W_Q = np.asarray(W_Q, dtype=np.float32)
    W_K = np.asarray(W_K, dtype=np.float32)
    W_V = np.asarray(W_V, dtype=np.float32)
    W_O = np.asarray(W_O, dtype=np.float32)
    cos, sin = _rope_tables()
    in_maps = []
    xTb = [np.ascontiguousarray(x[b].T).astype(bfnp) for b in range(B)]
    for b in range(B):
        for t in range(TP):
            qheads = list(range(HQ * t, HQ * t + HQ))
            kvheads = [HKV * t + i for i in range(HKV)]
            idxq = [d * HEADS + h for h in qheads for d in range(HD)]
            idxkv = [d * KV + kv for kv in kvheads for d in range(HD)]
            rows_o = [h * HD + d for h in qheads for d in range(HD)]
            in_maps.append(dict(
                xT=xTb[b],
                wq=np.ascontiguousarray(W_Q[idxq, :].T).astype(bfnp),
                wk=np.ascontiguousarray(W_K[idxkv, :].T).astype(bfnp),
                wv=np.ascontiguousarray(W_V[idxkv, :].T).astype(bfnp),
                wo=np.ascontiguousarray(W_O[:, rows_o].T).astype(bfnp),
                cosT=cos,
                sinT=sin,
            ))
    return in_maps


def emulate_core(m):
    """Numpy emulation of the device math for one core's in_map."""
    xT = np.asarray(m["xT"], np.float32)      # (E, S)
    wq = np.asarray(m["wq"], np.float32)      # (E, 512)
    wk = np.asarray(m["wk"], np.float32)
    wv = np.asarray(m["wv"], np.float32)
    wo = np.asarray(m["wo"], np.float32)      # (512, E)
    cos = np.asarray(m["cosT"], np.float32)   # (128, S)
    sin = np.asarray(m["sinT"], np.float32)

    def bfq(a):
        return a.astype(bfnp).astype(np.float32)

    qT = bfq(wq.T @ xT)                       # (512, S)
    kT = bfq(wk.T @ xT)
    vT = bfq(wv.T @ xT)

    def rope(blkT):  # (128, S)
        xw = np.concatenate([blkT[64:], blkT[:64]], axis=0)
        return bfq(blkT * cos + xw * sin)

    ctxs = []
    for h in range(HQ):
        qh = rope(qT[h * 128:(h + 1) * 128])
        kvl = h // 2
        kh = rope(kT[kvl * 128:(kvl + 1) * 128])
        vh = vT[kvl * 128:(kvl + 1) * 128]
        scoresT = kh.T @ qh * SCALE           # (t, s)
        w = bfq(np.exp(scoresT))
        den = w.sum(axis=0)
        ctxT = bfq((vh @ w) / den[None, :])
        ctxs.append(ctxT)
    ctx = np.concatenate(ctxs, axis=0)        # (512, S)
    return bfq(ctx.T @ wo)


def combine_outs(outs):
    out = np.empty((B, S, EMB), dtype=np.float32)
    for b in range(B):
        acc = np.asarray(outs[TP * b]).astype(np.float32)
        for t in range(1, TP):
            acc = acc + np.asarray(outs[TP * b + t]).astype(np.float32)
        out[b] = acc
    return out


LAST_RESULTS = None


def kernel(x, W_Q, W_K, W_V, W_O):
    global LAST_RESULTS
    from concourse.bass_utils import run_bass_kernel_spmd

    nc = _get_nc()
    in_maps = build_in_maps(x, W_Q, W_K, W_V, W_O)
    res = run_bass_kernel_spmd(nc, in_maps, list(range(NCORES)))
    LAST_RESULTS = res
    outs = [r["out"] for r in res.results]
    return combine_outs(outs)
